# revision 31
# baseline (speedup 1.0000x reference)
"""Trainium2 Bass kernel for nn_CausalVAE (encoder MLP + reparam + 32-node
sequential causal decoder).

Sharding: data-parallel over batch across 8 NeuronCores (4096 rows/core),
weights replicated. On-chip layout is feature-major (features on SBUF
partitions, batch on the free dim) so every layer is a chain of
[K<=128, M<=128] x [K, 512] fp32r matmuls with no on-chip transposes
(inputs/outputs are transposed on the host as part of shard/gather).
The per-node causal masking is exact: node i's first matmul reads only
the first i+1 partitions of the running y^T state tile.

Key device-side structure:
- b3 is folded out of the per-node epilogue on the HOST (decoded rows live
  without b3; the deficit is a per-node constant folded into the relu1
  bias b1), so the per-node epilogue is a single psum->sbuf row DMA;
- all DMAs share one FIFO (SP queue -> descriptor gen -> DMA engines), so
  ordering is everything: small weights are pre-packed on the host into
  their on-chip layouts (contiguous descriptors), W2 streams per-node from
  inside the decoder loop, and mu/lv output DMAs are spread across early
  loop steps so no bulk transfer ever sits in front of a row DMA the next
  node depends on;
- node 31 rows bypass the yT state and stream straight to the output, and
  per-tile output DMAs for rows 0..30 are emitted inside the pipeline loop
  so the tail is just the last row's DMA;
- the whole decoder is emitted as a 4-deep software pipeline because
  engine queues execute in emission order.

Matmul precision switchable via VAE_MM_MODE: float32r (default, tf32-like,
full speed) or float32 (exact, 4x slower).
"""

import os

import numpy as np

import concourse.bass as bass
import concourse.mybir as mybir
import concourse.tile as tile
from concourse import bacc
from concourse.alu_op_type import AluOpType
from concourse.bass import ts
from concourse.bass_utils import run_bass_kernel_spmd

D = 32          # causal nodes / feature dim of y
DF = 256        # hidden dim
B = 32768       # full batch
NCORES = 8
BL = B // NCORES          # 4096 rows per core
BT = 512                  # batch tile (matmul moving free dim)
NBT = BL // BT            # 8 batch tiles per core

F32 = mybir.dt.float32
F32R = mybir.dt.float32r
AF = mybir.ActivationFunctionType

# "float32r" (fast, tf32-like) or "float32" (exact, 4x slower matmul)
MM_MODE = os.environ.get("VAE_MM_MODE", "float32r")
# decoded rows: DMA straight from PSUM (1) or stage through SBUF (0)
SROW_DMA = os.environ.get("VAE_SROW_DMA", "0") == "1"


def _make_nc():
    rmode = MM_MODE == "float32r"

    def r(ap):
        """View an AP as float32r (matmul operands + their producers)."""
        return ap.bitcast(F32R) if rmode else ap

    nc = bacc.Bacc("TRN2", target_bir_lowering=False, debug=False)

    # ---- DRAM I/O (activations pre-transposed on host: [feat, batch]) ----
    # b1p/b2p/W3p/be1p/be2p are pre-packed on the host into the on-chip
    # layouts (b1p also carries the b3 fold, see _run).
    xorT_d = nc.dram_tensor("xorT", [D, BL], F32, kind="ExternalInput")
    epsT_d = nc.dram_tensor("epsT", [D, BL], F32, kind="ExternalInput")
    We1_d = nc.dram_tensor("We1", [D, DF], F32, kind="ExternalInput")
    be1p_d = nc.dram_tensor("be1p", [128, 2], F32, kind="ExternalInput")
    We2_d = nc.dram_tensor("We2", [DF, DF], F32, kind="ExternalInput")
    be2p_d = nc.dram_tensor("be2p", [128, 2], F32, kind="ExternalInput")
    We3_d = nc.dram_tensor("We3", [DF, 2 * D], F32, kind="ExternalInput")
    be3_d = nc.dram_tensor("be3", [2 * D], F32, kind="ExternalInput")
    W1_d = nc.dram_tensor("W1", [D, D, DF], F32, kind="ExternalInput")
    b1p_d = nc.dram_tensor("b1p", [128, D, 2], F32, kind="ExternalInput")
    W2_d = nc.dram_tensor("W2", [D, DF, DF], F32, kind="ExternalInput")
    b2p_d = nc.dram_tensor("b2p", [128, D, 2], F32, kind="ExternalInput")
    W3p_d = nc.dram_tensor("W3p", [128, D, 2], F32, kind="ExternalInput")
    yT_dr = nc.dram_tensor("yT", [D, BL], F32, kind="ExternalOutput")
    # the encoder streams out et=exp(lv/2) and y0 instead of mu/lv; the
    # host reconstructs lv = 2*ln(et), mu = y0 - et*eps exactly. This keeps
    # the whole reparam staging to one Act op + two DVE ops per tile.
    y0T_d = nc.dram_tensor("y0T", [D, BL], F32, kind="ExternalOutput")
    etT_d = nc.dram_tensor("etT", [D, BL], F32, kind="ExternalOutput")

    with tile.TileContext(nc) as tc:
        with (
            tc.tile_pool(name="wpool", bufs=1) as wp,
            tc.tile_pool(name="actp", bufs=1) as actp,
            tc.tile_pool(name="psum", bufs=6, space="PSUM") as psp,
            tc.tile_pool(name="psum3", bufs=2, space="PSUM") as ps3,
        ):
            # ---- persistent feature-major activations ----
            xorT = actp.tile([D, BL], F32)       # xor^T
            epsT = actp.tile([D, BL], F32)       # eps^T
            etT = actp.tile([D, BL], F32)        # exp(lv/2)^T
            yT = [actp.tile([D, BT], F32, name=f"yT{b}") for b in range(NBT)]

            # ---- PE warm-up ----
            # The tensor engine p-state ramps to full clock only after ~3us
            # of continuous execution. Run throwaway matmuls on a zeroed
            # scratch tile from t~0 so the ramp completes while the first
            # input DMAs are still in flight.
            wmsc = wp.tile([1, 256], F32)
            # Pool-engine memset: starts ~60ns in (DVE's first op pays ~700ns
            # of decode/queue latency; Pool memsets are proven in the
            # framework preamble)
            nc.gpsimd.memset(wmsc[:], 0.0)
            # dummy activation: hoists the one-time 1.3us activation-table
            # load off the encoder critical path, into the DMA wait
            wmact = wp.tile([1, 256], F32)
            nc.scalar.activation(wmact[:], wmsc[:], AF.Relu)
            for _ in range(8):
                wmps = ps3.tile([128, BT], F32, tag="p3", name="wmps")
                nc.tensor.matmul(wmps[:, 0:256], r(wmsc[:, 0:128]), r(wmsc[:]),
                                 start=True, stop=True)

            # ---- DMAs ordered by first use (startup-critical first) ----
            We1sb = wp.tile([D, DF], F32)
            nc.sync.dma_start(out=r(We1sb[:]), in_=r(We1_d[:]))
            nc.sync.dma_start(out=r(xorT[:, 0:BL // 4]),
                              in_=r(xorT_d[:, 0:BL // 4]))
            be1sb = wp.tile([128, 2], F32)
            nc.sync.dma_start(out=be1sb[:], in_=be1p_d[:])
            We2sb = wp.tile([128, 2, DF], F32)
            nc.sync.dma_start(out=r(We2sb[:]),
                              in_=r(We2_d[:].rearrange("(k p) c -> p k c", k=2)))
            We3sb = wp.tile([128, 2, 2 * D], F32)
            nc.sync.dma_start(out=r(We3sb[:]),
                              in_=r(We3_d[:].rearrange("(k p) c -> p k c", k=2)))
            be2sb = wp.tile([128, 2], F32)
            nc.sync.dma_start(out=be2sb[:], in_=be2p_d[:])
            # be3x: [be3_mu ; be3_lv/2] host-packed — the lv half is halved so
            # et = Exp(pz*0.5 + be3x) computes exp((pz+be3_lv)/2) straight
            # from psum
            be3sb = wp.tile([2 * D, 1], F32)
            nc.sync.dma_start(out=be3sb[:], in_=be3_d[:].unsqueeze(1))
            # interleave the remaining input chunks with the decoder weights,
            # ordered by first-use time (the DMA FIFO is strictly serial)
            w1pa = wp.tile([D, D, 128], F32)
            w1pb = wp.tile([D, D, 128], F32)
            b1sb = wp.tile([128, D, 2], F32)
            b2sb = wp.tile([128, D, 2], F32)
            W3sb = wp.tile([128, D, 2], F32)
            W2sb = wp.tile([128, D, 2, DF], F32)

            def w2_load(i):
                nc.sync.dma_start(
                    out=r(W2sb[:, i, :, :]),
                    in_=r(W2_d[i].rearrange("(k p) c -> p k c", k=2)))

            def eps_load(c):
                cs = ts(c, BL // 4)
                nc.sync.dma_start(out=epsT[:, cs], in_=epsT_d[:, cs])

            def xor_load(c):
                cs = ts(c, BL // 4)
                nc.sync.dma_start(out=r(xorT[:, cs]), in_=r(xorT_d[:, cs]))

            eps_load(0)
            xor_load(1)
            eps_load(1)
            # L1 weights for nodes 0-7 first (decoder starts mid-encoder)
            nc.sync.dma_start(out=r(w1pa[:, 0:8, :]),
                              in_=r(W1_d[0:8, :, 0:128].rearrange("i k c -> k i c")))
            nc.sync.dma_start(out=r(w1pb[:, 0:8, :]),
                              in_=r(W1_d[0:8, :, 128:256].rearrange("i k c -> k i c")))
            nc.sync.dma_start(out=b1sb[:], in_=b1p_d[:])
            nc.sync.dma_start(out=b2sb[:], in_=b2p_d[:])
            nc.sync.dma_start(out=r(W3sb[:]), in_=r(W3p_d[:]))
            w2_load(0)
            xor_load(2)
            eps_load(2)
            w2_load(1)
            xor_load(3)
            eps_load(3)
            # w1p nodes 8:32 and W2 nodes 2+ stream from inside the loop

            with (
                tc.tile_pool(name="hid1", bufs=3) as h1p,
                tc.tile_pool(name="hid2", bufs=3) as h2p,
                tc.tile_pool(name="smallp", bufs=2) as smp,
            ):
                # ---- encoder, feature-major, 1-step software pipeline ----
                enc_st = {}

                def enc_mm(bt):
                    bs = ts(bt, BT)
                    p1a = psp.tile([128, BT], F32, tag="ps", name="p1a")
                    nc.tensor.matmul(p1a[:], r(We1sb[:, 0:128]), r(xorT[:, bs]),
                                     start=True, stop=True)
                    p1b = psp.tile([128, BT], F32, tag="ps", name="p1b")
                    nc.tensor.matmul(p1b[:], r(We1sb[:, 128:256]), r(xorT[:, bs]),
                                     start=True, stop=True)
                    enc_st[bt] = (p1a, p1b)

                def enc_relu(bt):
                    p1a, p1b = enc_st.pop(bt)
                    h1a = h1p.tile([128, BT], F32, tag="t1a", name="h1a")
                    nc.scalar.activation(r(h1a[:]), p1a[:], AF.Relu, bias=be1sb[:, 0:1])
                    h1b = h1p.tile([128, BT], F32, tag="t1b", name="h1b")
                    nc.vector.tensor_scalar(r(h1b[:]), p1b[:], be1sb[:, 1:2], 0.0,
                                            AluOpType.add, AluOpType.max)
                    enc_st[bt] = (h1a, h1b)

                def enc_back(bt):
                    bs = ts(bt, BT)
                    h1a, h1b = enc_st.pop(bt)
                    p2a = psp.tile([128, BT], F32, tag="ps", name="p2a")
                    nc.tensor.matmul(p2a[:], r(We2sb[:, 0, 0:128]), r(h1a[:]),
                                     start=True, stop=False)
                    nc.tensor.matmul(p2a[:], r(We2sb[:, 1, 0:128]), r(h1b[:]),
                                     start=False, stop=True)
                    p2b = psp.tile([128, BT], F32, tag="ps", name="p2b")
                    nc.tensor.matmul(p2b[:], r(We2sb[:, 0, 128:256]), r(h1a[:]),
                                     start=True, stop=False)
                    nc.tensor.matmul(p2b[:], r(We2sb[:, 1, 128:256]), r(h1b[:]),
                                     start=False, stop=True)
                    h2a = h2p.tile([128, BT], F32, tag="t2a", name="h2a")
                    nc.scalar.activation(r(h2a[:]), p2a[:], AF.Relu, bias=be2sb[:, 0:1])
                    # h2b also on Act: the encoder is DVE-bound (reparam chain
                    # below), Act has the headroom
                    h2b = h2p.tile([128, BT], F32, tag="t2b", name="h2b")
                    nc.scalar.activation(r(h2b[:]), p2b[:], AF.Relu, bias=be2sb[:, 1:2])
                    pz = psp.tile([2 * D, BT], F32, tag="ps", name="pz")
                    nc.tensor.matmul(pz[:], r(We3sb[:, 0, :]), r(h2a[:]),
                                     start=True, stop=False)
                    nc.tensor.matmul(pz[:], r(We3sb[:, 1, :]), r(h2b[:]),
                                     start=False, stop=True)
                    # et = exp((lv_raw+be3_lv)/2) straight from psum (be3x lv
                    # rows are pre-halved); y0 = (mu_raw + be3_mu) + et*eps
                    nc.scalar.activation(etT[:, bs], pz[D:2 * D, :], AF.Exp,
                                         scale=0.5, bias=be3sb[D:2 * D, :])
                    tm = smp.tile([D, BT], F32, tag="tm", name="tm")
                    nc.vector.tensor_mul(tm[:], etT[:, bs], epsT[:, bs])
                    nc.vector.scalar_tensor_tensor(
                        r(yT[bt][:, :]), pz[0:D, :], be3sb[0:D, :], tm[:],
                        AluOpType.add, AluOpType.add)

                # (the encoder loop is below, merged with the decoder start)

                # ---- sequential causal decoder ----
                # Software-pipelined emission: the in-order engine queues
                # preserve emission order, so interleaving stages of
                # consecutive iterations is what lets iteration k+1's L1 run
                # while k's relu/L2 are still in flight.
                NIT = D * NBT
                st1, st2, st3, st4 = {}, {}, {}, {}

                def stage_l1(k):
                    i, b = divmod(k, NBT)
                    ke = i + 1
                    p1a = psp.tile([128, BT], F32, tag="ps", name="p1a")
                    nc.tensor.matmul(p1a[:], r(w1pa[0:ke, i, :]),
                                     r(yT[b][0:ke, :]), start=True, stop=True)
                    p1b = psp.tile([128, BT], F32, tag="ps", name="p1b")
                    nc.tensor.matmul(p1b[:], r(w1pb[0:ke, i, :]),
                                     r(yT[b][0:ke, :]), start=True, stop=True)
                    st1[k] = (i, b, p1a, p1b)

                def stage_relu1(k):
                    i, b, p1a, p1b = st1.pop(k)
                    t1a = h1p.tile([128, BT], F32, tag="t1a", name="t1a")
                    nc.scalar.activation(r(t1a[:]), p1a[:], AF.Relu,
                                         bias=b1sb[:, i, 0:1])
                    t1b = h1p.tile([128, BT], F32, tag="t1b", name="t1b")
                    nc.vector.tensor_scalar(r(t1b[:]), p1b[:], b1sb[:, i, 1:2],
                                            0.0, AluOpType.add, AluOpType.max)
                    st2[k] = (i, b, t1a, t1b)

                def stage_l2(k):
                    i, b, t1a, t1b = st2.pop(k)
                    p2a = psp.tile([128, BT], F32, tag="ps", name="p2a")
                    nc.tensor.matmul(p2a[:], r(W2sb[:, i, 0, 0:128]), r(t1a[:]),
                                     start=True, stop=False)
                    nc.tensor.matmul(p2a[:], r(W2sb[:, i, 1, 0:128]), r(t1b[:]),
                                     start=False, stop=True)
                    p2b = psp.tile([128, BT], F32, tag="ps", name="p2b")
                    nc.tensor.matmul(p2b[:], r(W2sb[:, i, 0, 128:256]), r(t1a[:]),
                                     start=True, stop=False)
                    nc.tensor.matmul(p2b[:], r(W2sb[:, i, 1, 128:256]), r(t1b[:]),
                                     start=False, stop=True)
                    st3[k] = (i, b, p2a, p2b)

                def stage_relu2(k):
                    i, b, p2a, p2b = st3.pop(k)
                    t2a = h2p.tile([128, BT], F32, tag="t2a", name="t2a")
                    nc.scalar.activation(r(t2a[:]), p2a[:], AF.Relu,
                                         bias=b2sb[:, i, 0:1])
                    t2b = h2p.tile([128, BT], F32, tag="t2b", name="t2b")
                    nc.vector.tensor_scalar(r(t2b[:]), p2b[:], b2sb[:, i, 1:2],
                                            0.0, AluOpType.add, AluOpType.max)
                    st4[k] = (i, b, t2a, t2b)

                def stage_l3(k):
                    i, b, t2a, t2b = st4.pop(k)
                    p3 = ps3.tile([1, BT], F32, tag="p3", name="p3")
                    nc.tensor.matmul(p3[:], r(W3sb[:, i, 0:1]), r(t2a[:]),
                                     start=True, stop=False)
                    nc.tensor.matmul(p3[:], r(W3sb[:, i, 1:2]), r(t2b[:]),
                                     start=False, stop=True)
                    if SROW_DMA:
                        src = p3[:]
                    else:
                        srow = smp.tile([1, BT], F32, tag="srow", name="srow",
                                        bufs=3)
                        # ~60% of the row copies on Act, 40% on DVE: balances
                        # the two (DVE carries the relu-b chain + encoder
                        # reparam; Act carries relu-a + et)
                        if k % 5 >= 3:
                            nc.vector.tensor_copy(srow[:], p3[:])
                        else:
                            nc.scalar.activation(srow[:], p3[:], AF.Copy)
                        src = srow[:]
                    if i < D - 1:
                        nc.sync.dma_start(out=r(yT[b][i:i + 1, :]), in_=r(src))
                    else:
                        # node 31 feeds nothing downstream: stream the row
                        # straight to the output
                        nc.sync.dma_start(out=yT_dr[D - 1:D, ts(b, BT)],
                                          in_=src)

                def yfix_rows(b):
                    # rows 0..30 of tile b are final once node 30's row DMA
                    # lands: stream them out, overlapping node-31 compute
                    # (b3 is added back on the host at gather time)
                    nc.sync.dma_start(out=yT_dr[0:D - 1, ts(b, BT)],
                                      in_=yT[b][0:D - 1, :])

                # 4-deep pipeline: every PE stage consumes only results from
                # strictly earlier steps, so PE never waits on same-step
                # vector work.
                def dec_step(k):
                    if k < NIT:
                        stage_l1(k)
                        stage_relu1(k)
                    if 2 <= k < NIT + 2:
                        stage_relu2(k - 2)
                    if 3 <= k:
                        stage_l3(k - 3)
                    if 1 <= k < NIT + 1:
                        stage_l2(k - 1)
                    # widely-spaced bulk DMAs, between the row DMAs they
                    # must not delay. The y0/et outputs for tile b go at step
                    # b: the DMA FIFO then guarantees the y0 read of yT[b]
                    # row 0 completes before node 0's row DMA (step b+3)
                    # overwrites it.
                    if k < NBT:
                        bs = ts(k, BT)
                        nc.sync.dma_start(out=y0T_d[:, bs], in_=yT[k][:, :])
                        nc.sync.dma_start(out=etT_d[:, bs], in_=etT[:, bs])
                    if k == NBT + 1:
                        w2_load(2)
                    if k == NBT + 3:
                        w2_load(3)
                    if k == 2 * NBT:
                        nc.sync.dma_start(
                            out=r(w1pa[:, 8:D, :]),
                            in_=r(W1_d[8:D, :, 0:128].rearrange("i k c -> k i c")))
                    if k == 2 * NBT + 2:
                        nc.sync.dma_start(
                            out=r(w1pb[:, 8:D, :]),
                            in_=r(W1_d[8:D, :, 128:256].rearrange("i k c -> k i c")))
                    if k < NIT and k % NBT == 5 and 4 <= k // NBT + 4 < D:
                        w2_load(k // NBT + 4)
                    if (D - 1) * NBT <= k < D * NBT:
                        yfix_rows(k - (D - 1) * NBT)

                # encoder loop, with the first decoder steps interleaved from
                # s=4 on: the encoder is DVE/Act-paced (reparam chain), so PE
                # has idle slots the early decoder matmuls can fill. Per-step
                # order: next tile's L1 matmuls first (keeps PE fed without
                # waiting on fresh relus), then the previous tile's back half
                # (so the reparam chain isn't queued behind the next tile's
                # relus), then this tile's relus.
                MERGE0 = 4
                for s in range(NBT + 1):
                    if s < NBT:
                        enc_mm(s)
                    if s >= 1:
                        enc_back(s - 1)
                    if s < NBT:
                        enc_relu(s)
                    if s >= MERGE0:
                        dec_step(s - MERGE0)
                for k in range(NBT + 1 - MERGE0, NIT + 3):
                    dec_step(k)

    nc.compile()
    return nc


_NC_CACHE = None


def _get_nc():
    global _NC_CACHE
    if _NC_CACHE is None:
        _NC_CACHE = _make_nc()
    return _NC_CACHE


def _pack_pdm(a):
    """[D, 256] -> [128, D, 2] with out[p, i, m] = a[i, m*128+p]."""
    return np.ascontiguousarray(
        a.reshape(D, 2, 128).transpose(2, 0, 1), dtype=np.float32)


def _run(inputs, trace=False):
    f32c = lambda a: np.ascontiguousarray(np.asarray(a), dtype=np.float32)
    xorT = f32c(inputs["xor"]).T   # [D, B]
    epsT = f32c(inputs["eps"]).T
    shared = {k: f32c(inputs[k]) for k in
              ["We1", "We2", "We3", "W1", "W2"]}
    # be3 packed as [be3_mu ; be3_lv/2] (lv half pre-halved for the fused
    # exp((lv+be3_lv)/2) activation)
    be3 = f32c(inputs["be3"])
    shared["be3"] = np.concatenate([be3[0:D], be3[D:2 * D] / 2.0])
    # Decoded y rows live on-device WITHOUT b3 (pure psum row DMAs);
    # node i's L1 input is then short by sum_{k<i} b3[k]*W1[i][k,:], a
    # weight-only constant folded into the relu1 bias here. b3 is added
    # back at output time. Small weights are pre-packed into their
    # on-chip layouts so their DMAs are contiguous.
    W1 = f32c(inputs["W1"]).astype(np.float64)
    b3 = f32c(inputs["b3"]).astype(np.float64)
    mask = np.tril(np.ones((D, D)), -1)
    corr = np.einsum("ik,k,ikc->ic", mask, b3, W1)
    b1mod = (f32c(inputs["b1"]).astype(np.float64) + corr).astype(np.float32)
    shared["b1p"] = _pack_pdm(b1mod)
    shared["b2p"] = _pack_pdm(f32c(inputs["b2"]))
    shared["W3p"] = _pack_pdm(f32c(inputs["W3"]))
    shared["be1p"] = np.ascontiguousarray(
        f32c(inputs["be1"]).reshape(2, 128).T)
    shared["be2p"] = np.ascontiguousarray(
        f32c(inputs["be2"]).reshape(2, 128).T)
    in_maps = []
    for c in range(NCORES):
        m = dict(shared)
        m["xorT"] = np.ascontiguousarray(xorT[:, c * BL:(c + 1) * BL])
        m["epsT"] = np.ascontiguousarray(epsT[:, c * BL:(c + 1) * BL])
        in_maps.append(m)
    nc = _get_nc()
    res = run_bass_kernel_spmd(nc, in_maps, core_ids=list(range(NCORES)),
                               trace=trace)
    gather = lambda nm: np.ascontiguousarray(
        np.concatenate([r[nm] for r in res.results], axis=1).T)
    # decoded rows come back without b3 (folded out on-device); add it here.
    # mu/lv are reconstructed exactly from the device's et=exp(lv/2) and
    # y0 = mu + et*eps streams.
    y = gather("yT") + f32c(inputs["b3"])[None, :]
    et = gather("etT").astype(np.float64)
    y0 = gather("y0T").astype(np.float64)
    lv = (2.0 * np.log(et)).astype(np.float32)
    mu = (y0 - et * f32c(inputs["eps"]).astype(np.float64)).astype(np.float32)
    return (y, mu, lv, y), res


def kernel(**inputs):
    out, _ = _run(inputs)
    return out
